# revision 1
# baseline (speedup 1.0000x reference)
"""Bass/Trainium2 kernel for nn_BiLSTM_Tok_83837761618147.

Strategy (8 NeuronCores, SPMD, full inputs in / full output out):
  - Token dim sharded 8 ways (16384 tokens/core, with halos).
  - BiLSTM parallelized via chunked recurrence with burn-in: each core runs
    128 lanes x (128+64) steps forward and 128 lanes x (129+64) steps
    backward (state forgets exponentially; 64 warmup steps reach fp32
    accuracy; the true h0/c0-seeded lanes cover the sequence ends exactly).
  - Gate pre-activations computed by PE matmuls directly into PSUM
    (bias via a K=4 indicator matmul); w_hh @ h accumulated on top.
  - Attention (tanh/logits/exp) + ragged segment softmax-sum done on
    device via an e-weighted one-hot (token x segment-window) matmul.
  - Host combines per-core partial [segment, 257] sums, normalizes, and
    applies the tiny tag projection.
"""

import numpy as np
import ml_dtypes

BF16 = ml_dtypes.bfloat16

T = 131072
D = 256
H = 128
HID = 256
TAGS = 10
S = 1024
NCORE = 8
PC = T // NCORE          # 16384 tokens per core
B = 64                   # burn-in steps
LF = 128                 # forward lane length (tokens per lane)
LB = 129                 # backward lane length
NL = 128                 # lanes per direction
NSF = B + LF             # 192 forward steps
NSB = B + LB             # 193 backward steps
SH = 16640               # x shard rows [tc0-64, tc0-64+SH)
SWIN = 256               # segment window width per core
NQ = PC                  # main attention window positions
NTILE = NQ // 128        # 128 main token tiles
HBW = LB * NL - LB + LB + B  # hbT width: 16512
HBT_W = 16512
ATT_W = NQ + 128         # att buffer width (main + extra tile)

_BUILT = {}
LAST_RESULT = None


def _build():
    if "nc" in _BUILT:
        return _BUILT["nc"]
    import contextlib
    from concourse import bacc, mybir
    from concourse.tile import TileContext

    F32 = mybir.dt.float32
    BF = mybir.dt.bfloat16
    AF = mybir.ActivationFunctionType
    ALU = mybir.AluOpType

    nc = bacc.Bacc()

    def din(name, shape, dt):
        return nc.declare_dram_parameter(name, list(shape), dt, isOutput=False)

    x_in = din("xT", [256, SH], BF)
    wih_f_in = din("wih_f", [256, 512], BF)
    wih_b_in = din("wih_b", [256, 512], BF)
    whh_f_in = din("whh_f", [128, 512], BF)
    whh_b_in = din("whh_b", [128, 512], BF)
    b4_f_in = din("b4_f", [128, 128], BF)
    b4_b_in = din("b4_b", [128, 128], BF)
    i4_in = din("i4", [128, 1024], BF)
    h0f_in = din("h0f", [128, 128], BF)
    c0f_in = din("c0f", [128, 128], BF)
    h0b_in = din("h0b", [128, 128], BF)
    c0b_in = din("c0b", [128, 128], BF)
    wom_in = din("wom", [256, 256], BF)
    uo_in = din("uo", [256, 1], BF)
    iota_in = din("iota", [128, 256], F32)
    identb_in = din("identb", [128, 128], BF)
    identf_in = din("identf", [128, 128], F32)
    seg_in = din("seg", [128, 129], F32)
    ctx_out = nc.declare_dram_parameter("ctx", [256, 257], F32, isOutput=True)
    att_dram = nc.dram_tensor("att_stage", [1, ATT_W], F32)

    with TileContext(nc) as tc, contextlib.ExitStack() as ctx:
        pp = ctx.enter_context(tc.tile_pool(name="persist", bufs=1))

        xT0 = pp.tile([128, SH], BF, tag="xT0", name="xT0")
        xT1 = pp.tile([128, SH], BF, tag="xT1", name="xT1")
        hfT = pp.tile([128, NQ], BF, tag="hfT", name="hfT")
        hbT = pp.tile([128, HBT_W], BF, tag="hbT", name="hbT")
        hf_head = pp.tile([128, 64], BF, tag="hfh", name="hfh")
        hb_head = pp.tile([128, 64], BF, tag="hbh", name="hbh")
        scr = [[pp.tile([128, 128], BF, tag=f"scr{d}{i}", name=f"scr{d}{i}") for i in range(2)]
               for d in range(2)]
        wih = [[pp.tile([128, 512], BF, tag=f"wih{d}{k}", name=f"wih{d}{k}") for k in range(2)]
               for d in range(2)]
        whh = [pp.tile([128, 512], BF, tag=f"whh{d}", name=f"whh{d}") for d in range(2)]
        b4 = [pp.tile([128, 128], BF, tag=f"b4{d}", name=f"b4{d}") for d in range(2)]
        i4 = pp.tile([128, 1024], BF, tag="i4", name="i4")
        h0 = [pp.tile([128, 128], BF, tag=f"h0{d}", name=f"h0{d}") for d in range(2)]
        c0 = [pp.tile([128, 128], BF, tag=f"c0{d}", name=f"c0{d}") for d in range(2)]
        wom = [pp.tile([128, 256], BF, tag=f"wom{k}", name=f"wom{k}") for k in range(2)]
        uo = [pp.tile([128, 1], BF, tag=f"uo{k}", name=f"uo{k}") for k in range(2)]
        iota_t = pp.tile([128, 256], F32, tag="iota", name="iota")
        identb = pp.tile([128, 128], BF, tag="identb", name="identb")
        identf = pp.tile([128, 128], F32, tag="identf", name="identf")
        seg_t = pp.tile([128, 129], F32, tag="seg", name="seg")
        CFB = pp.tile([128, 256], BF, tag="CFB", name="CFB")
        e_cm = pp.tile([128, 129], F32, tag="ecm", name="ecm")
        hfx = pp.tile([128, 128], BF, tag="hfx", name="hfx")
        hbx = pp.tile([128, 128], BF, tag="hbx", name="hbx")
        ctx_sb = [pp.tile([128, 257], F32, tag=f"ctxsb{k}", name=f"ctxsb{k}") for k in range(2)]

        # ---- input DMAs ----
        nc.sync.dma_start(xT0[:], x_in[0:128, :])
        nc.sync.dma_start(xT1[:], x_in[128:256, :])
        for d, t_ in ((0, wih_f_in), (1, wih_b_in)):
            nc.sync.dma_start(wih[d][0][:], t_[0:128, :])
            nc.sync.dma_start(wih[d][1][:], t_[128:256, :])
        nc.sync.dma_start(whh[0][:], whh_f_in[:])
        nc.sync.dma_start(whh[1][:], whh_b_in[:])
        nc.sync.dma_start(b4[0][:], b4_f_in[:])
        nc.sync.dma_start(b4[1][:], b4_b_in[:])
        nc.sync.dma_start(i4[:], i4_in[:])
        nc.sync.dma_start(h0[0][:], h0f_in[:])
        nc.sync.dma_start(c0[0][:], c0f_in[:])
        nc.sync.dma_start(h0[1][:], h0b_in[:])
        nc.sync.dma_start(c0[1][:], c0b_in[:])
        nc.sync.dma_start(wom[0][:], wom_in[0:128, :])
        nc.sync.dma_start(wom[1][:], wom_in[128:256, :])
        nc.sync.dma_start(uo[0][:], uo_in[0:128, :])
        nc.sync.dma_start(uo[1][:], uo_in[128:256, :])
        nc.sync.dma_start(iota_t[:], iota_in[:])
        nc.sync.dma_start(identb[:], identb_in[:])
        nc.sync.dma_start(identf[:], identf_in[:])
        nc.sync.dma_start(seg_t[:], seg_in[:])

        # init cell state from seeds: CFB = [c0f | c0b]
        nc.vector.tensor_copy(CFB[:, 0:128], c0[0][:])
        nc.vector.tensor_copy(CFB[:, 128:256], c0[1][:])

        xT = [xT0, xT1]

        def fwd_pre_rhs(kh, s0):
            # cols {128m + 64 + s0 + ds}, ds in {0,1}, m in [0,128)
            base = 64 + s0
            v = xT[kh][:, base:base + 16384]
            v = v.rearrange("p (m b) -> p b m", b=128)
            return v[:, 0:2, :]

        def bwd_pre_rhs(kh, s):
            # backward lane k' reads x col 193 + 129*k' - s, k' in [0,128)
            a = 193 - s
            return xT[kh][:, a:a + 129 * 127 + 1:129]

        def h_src(d, s):
            # h state produced at step s-1 (read at step s)
            if s == 0:
                return h0[d][:]
            sp = s - 1
            if sp < B:
                return scr[d][sp % 2][:]
            if d == 0:
                return hfT[:, sp - 64:sp - 64 + 127 * 128 + 1:128]
            a = 192 - sp
            return hbT[:, a:a + 129 * 127 + 1:129]

        def h_dst(d, s):
            if s < B:
                return scr[d][s % 2][:]
            if d == 0:
                if s == 192:
                    return scr[0][0][:]
                return hfT[:, s - 64:s - 64 + 127 * 128 + 1:128]
            a = 192 - s
            return hbT[:, a:a + 129 * 127 + 1:129]

        with tc.tile_pool(name="psG", bufs=4, space="PSUM") as psg, \
             tc.tile_pool(name="sig", bufs=3) as sigp, \
             tc.tile_pool(name="tg", bufs=3) as tgp, \
             tc.tile_pool(name="tcn", bufs=3) as tcp, \
             tc.tile_pool(name="tmp1", bufs=3) as t1p, \
             tc.tile_pool(name="tmp2", bufs=3) as t2p:

            G = {}

            def emit_pre(s_):
                # pre-gate + bias matmuls for step s_ (both dirs)
                if s_ >= NSB:
                    return
                g = psg.tile([128, 1024], F32, tag="G", name="G")
                G[s_] = g
                for d in range(2):
                    do = d * 512
                    nc.tensor.matmul(g[:, do:do + 512], b4[d][:], i4[:, 0:512],
                                     start=True, stop=False)
                    for kh in range(2):
                        for j in range(4):
                            if d == 0:
                                if s_ >= NSF:
                                    continue
                                base = 64 + s_
                                rhs = xT[kh][:, base:base + 127 * 128 + 1:128]
                            else:
                                rhs = bwd_pre_rhs(kh, s_)
                            nc.tensor.matmul(
                                g[:, do + 128 * j:do + 128 * j + 128],
                                wih[d][kh][:, 128 * j:128 * j + 128],
                                rhs, start=False, stop=False)

            for s_ in range(3):
                emit_pre(s_)

            for s in range(NSB):
                g = G.pop(s)
                emit_pre(s + 3)
                # w_hh matmuls (accumulate into this step's gate region)
                for d in range(2):
                    if d == 0 and s >= NSF:
                        continue
                    hs = h_src(d, s)
                    for j in range(4):
                        nc.tensor.matmul(
                            g[:, 512 * d + 128 * j:512 * d + 128 * j + 128],
                            whh[d][:, 128 * j:128 * j + 128], hs,
                            start=False, stop=True)
                # gates
                sig = sigp.tile([128, 768], BF, tag="sig", name="sig")
                src_sig = g[:].rearrange("p (a q) -> p a q", q=512)[:, :, 0:384]
                dst_sig = sig[:].rearrange("p (a q) -> p a q", q=384)
                nc.scalar.activation(dst_sig, src_sig, AF.Sigmoid)
                tg = tgp.tile([128, 256], BF, tag="tg", name="tg")
                src_tg = g[:].rearrange("p (a q) -> p a q", q=512)[:, :, 384:512]
                nc.scalar.activation(tg[:].rearrange("p (a q) -> p a q", q=128),
                                     src_tg, AF.Tanh)
                # c update
                sigr = sig[:].rearrange("p (a q) -> p a q", q=384)
                t1 = t1p.tile([128, 256], BF, tag="t1", name="t1")
                t2 = t2p.tile([128, 256], BF, tag="t2", name="t2")
                cr = CFB[:].rearrange("p (a q) -> p a q", q=128)
                nc.vector.tensor_tensor(t1[:].rearrange("p (a q) -> p a q", q=128),
                                        sigr[:, :, 128:256], cr, ALU.mult)
                nc.vector.tensor_tensor(t2[:].rearrange("p (a q) -> p a q", q=128),
                                        sigr[:, :, 0:128],
                                        tg[:].rearrange("p (a q) -> p a q", q=128),
                                        ALU.mult)
                nc.vector.tensor_tensor(CFB[:], t1[:], t2[:], ALU.add)
                tcn = tcp.tile([128, 256], BF, tag="tcn", name="tcn")
                nc.scalar.activation(tcn[:], CFB[:], AF.Tanh)
                # h = sigma_o * tanh(c), written straight to its storage slot
                for d in range(2):
                    if d == 0 and s >= NSF:
                        continue
                    nc.vector.tensor_tensor(h_dst(d, s),
                                            sig[:, 384 * d + 256:384 * d + 384],
                                            tcn[:, 128 * d:128 * d + 128],
                                            ALU.mult)
                if s < B:
                    nc.vector.tensor_copy(hf_head[:, s:s + 1],
                                          scr[0][s % 2][:, 0:1])
                    nc.vector.tensor_copy(hb_head[:, 63 - s:64 - s],
                                          scr[1][s % 2][:, 126:127])

        # ---------------- attention phase ----------------
        # assemble extra window tiles
        nc.vector.tensor_copy(hfx[:, 0:64], hf_head[:])
        nc.vector.tensor_copy(hfx[:, 64:128], hfT[:, 16256:16320])
        nc.vector.tensor_copy(hbx[:, 0:64], hbT[:, 63:127])
        nc.vector.tensor_copy(hbx[:, 64:128], hb_head[:])

        with tc.tile_pool(name="psU", bufs=2, space="PSUM") as psu, \
             tc.tile_pool(name="uT", bufs=2) as utp, \
             tc.tile_pool(name="psA", bufs=2, space="PSUM") as psa:
            for gidx in range(33):
                if gidx < 32:
                    n = 512
                    hfr = hfT[:, 512 * gidx:512 * gidx + 512]
                    hbr = hbT[:, 512 * gidx + 127:512 * gidx + 127 + 512]
                    aout = att_dram[0:1, 512 * gidx:512 * gidx + 512]
                else:
                    n = 128
                    hfr = hfx[:]
                    hbr = hbx[:]
                    aout = att_dram[0:1, NQ:NQ + 128]
                pa = psa.tile([1, 512], F32, tag="psA", name="psA")
                for c2 in range(2):
                    pu = psu.tile([128, 512], F32, tag="psU", name="psU")
                    nc.tensor.matmul(pu[:, 0:n], wom[0][:, 128 * c2:128 * c2 + 128],
                                     hfr, start=True, stop=False)
                    nc.tensor.matmul(pu[:, 0:n], wom[1][:, 128 * c2:128 * c2 + 128],
                                     hbr, start=False, stop=True)
                    ut = utp.tile([128, 512], BF, tag="uT", name="uT")
                    nc.scalar.activation(ut[:, 0:n], pu[:, 0:n], AF.Tanh)
                    nc.tensor.matmul(pa[0:1, 0:n], uo[c2][:], ut[:, 0:n],
                                     start=(c2 == 0), stop=(c2 == 1))
                asb = utp.tile([1, 512], F32, tag="asb", name="asb")
                nc.vector.tensor_copy(asb[0:1, 0:n], pa[0:1, 0:n])
                nc.sync.dma_start(aout, asb[0:1, 0:n])

        # att -> column-major e
        with tc.tile_pool(name="psT", bufs=2, space="PSUM") as pst, \
             tc.tile_pool(name="anm", bufs=1) as anmp:
            att_nm = anmp.tile([128, 128], F32, tag="anm", name="anm")
            nc.sync.dma_start(
                att_nm[:],
                att_dram[0:1, 0:NQ].rearrange("a (n p) -> (a n) p", p=128))
            ps_a = pst.tile([128, 128], F32, tag="psT", name="psT")
            nc.tensor.transpose(ps_a[:], att_nm[:], identf[:])
            nc.scalar.activation(e_cm[:, 0:128], ps_a[:], AF.Exp)
            att_x = anmp.tile([128, 1], F32, tag="attx", name="attx")
            nc.sync.dma_start(
                att_x[:],
                att_dram[0:1, NQ:NQ + 128].rearrange("a (n p) -> (a n) p", p=1))
            nc.scalar.activation(e_cm[:, 128:129], att_x[:], AF.Exp)

        # ragged context accumulation
        with tc.tile_pool(name="psT2", bufs=2, space="PSUM") as pst2, \
             tc.tile_pool(name="yp", bufs=2) as yp, \
             tc.tile_pool(name="iw", bufs=2) as iwp, \
             tc.tile_pool(name="psC", bufs=1, space="PSUM") as psc:
            ctxp = [psc.tile([128, 257], F32, tag=f"ctxp{k}", name=f"ctxp{k}") for k in range(2)]
            for nti in range(NTILE + 1):
                if nti < NTILE:
                    hfr = hfT[:, 128 * nti:128 * nti + 128]
                    hbr = hbT[:, 128 * nti + 127:128 * nti + 255]
                else:
                    hfr = hfx[:]
                    hbr = hbx[:]
                ps_t = pst2.tile([128, 256], BF, tag="psT2", name="psT2")
                nc.tensor.transpose(ps_t[:, 0:128], hfr, identb[:])
                nc.tensor.transpose(ps_t[:, 128:256], hbr, identb[:])
                y = yp.tile([128, 257], BF, tag="y", name="y")
                nc.vector.tensor_copy(y[:, 0:256], ps_t[:])
                nc.vector.memset(y[:, 256:257], 1.0)
                iw = iwp.tile([128, 256], BF, tag="iw", name="iw")
                nc.vector.tensor_scalar(iw[:], iota_t[:],
                                        seg_t[:, nti:nti + 1],
                                        e_cm[:, nti:nti + 1],
                                        ALU.is_equal, ALU.mult)
                for k in range(2):
                    nc.tensor.matmul(ctxp[k][:], iw[:, 128 * k:128 * k + 128],
                                     y[:], start=(nti == 0), stop=(nti == NTILE))
            for k in range(2):
                nc.vector.tensor_copy(ctx_sb[k][:], ctxp[k][:])
        for k in range(2):
            nc.sync.dma_start(ctx_out[128 * k:128 * k + 128, :], ctx_sb[k][:])

    nc.finalize()
    _BUILT["nc"] = nc
    return nc


def _host_prep(inputs):
    x = np.asarray(inputs["sentence"], np.float32)
    doc_mask = np.asarray(inputs["doc_mask"]).astype(np.int64)
    h0g = np.asarray(inputs["h0"], np.float32)
    c0g = np.asarray(inputs["c0"], np.float32)

    perm = np.r_[0:128, 128:256, 384:512, 256:384]  # i,f,o,g order

    def wprep(w):  # [4H, X] -> lhsT [X, 4H] with gate perm, bf16
        return np.ascontiguousarray(w.astype(np.float32).T[:, perm]).astype(BF16)

    wih = {d: wprep(np.asarray(inputs[f"w_ih_{s}"], np.float32))
           for d, s in ((0, "f"), (1, "b"))}
    whh = {d: wprep(np.asarray(inputs[f"w_hh_{s}"], np.float32))
           for d, s in ((0, "f"), (1, "b"))}
    bias = {d: (np.asarray(inputs[f"b_ih_{s}"], np.float32)
                + np.asarray(inputs[f"b_hh_{s}"], np.float32))[perm]
            for d, s in ((0, "f"), (1, "b"))}
    b4 = {}
    for d in range(2):
        m = np.zeros((128, 128), np.float32)
        for k in range(4):
            m[k, :] = bias[d][128 * k:128 * k + 128]
        b4[d] = m.astype(BF16)
    i4 = np.zeros((128, 1024), np.float32)
    for r in range(2):
        for k in range(4):
            i4[k, 512 * r + 128 * k: 512 * r + 128 * k + 128] = 1.0
    i4 = i4.astype(BF16)

    wom = np.asarray(inputs["w_omega"], np.float32).astype(BF16)
    uo = np.asarray(inputs["u_omega"], np.float32).astype(BF16)
    iota = np.tile(np.arange(256, dtype=np.float32), (128, 1))
    identb = np.eye(128, dtype=np.float32).astype(BF16)
    identf = np.eye(128, dtype=np.float32)

    seg_global = np.searchsorted(doc_mask, np.arange(T), side="right")

    in_maps = []
    s_los = []
    xpad = np.zeros((T + 512, D), np.float32)
    xpad[64:64 + T] = x  # global row r ↔ token r - 64
    for c in range(NCORE):
        tc0 = c * PC
        xs = xpad[tc0:tc0 + SH]  # token tc0-64+i at row i
        xT = np.ascontiguousarray(xs.T).astype(BF16)

        # seeds
        h0f = np.zeros((128, 128), np.float32)
        c0f = np.zeros((128, 128), np.float32)
        h0b = np.zeros((128, 128), np.float32)
        c0b = np.zeros((128, 128), np.float32)
        if c == 0:
            h0f[:, 0] = h0g[0]
            c0f[:, 0] = c0g[0]
        if c == NCORE - 1:
            h0b[:, 126] = h0g[1]
            c0b[:, 126] = c0g[1]

        # segment ids, col-major [128, 129]
        segm = np.full((128, 129), -1.0, np.float32)
        toks_main = tc0 + 64 + np.arange(NQ)
        valid = toks_main < T
        if c == NCORE - 1:
            valid &= (np.arange(NQ) < 16256)  # tail handled by W_tail
        toks_extra = np.full(128, -1, np.int64)
        if c == 0:
            toks_extra[0:64] = np.arange(64)          # W_head: tokens [0,64)
        if c == NCORE - 1:
            toks_extra[64:128] = T - 64 + np.arange(64)  # W_tail
        all_toks = np.concatenate([toks_main[valid],
                                   toks_extra[toks_extra >= 0]])
        s_lo = int(seg_global[all_toks].min()) if all_toks.size else 0
        s_hi = int(seg_global[all_toks].max()) if all_toks.size else 0
        assert s_hi - s_lo < SWIN, f"segment window too wide: {s_hi - s_lo}"
        s_los.append(s_lo)
        sm = np.where(valid, seg_global[np.minimum(toks_main, T - 1)] - s_lo,
                      -1.0).astype(np.float32)
        segm[:, 0:128] = sm.reshape(128, 128).T  # segm[p, n] = seg(q=128n+p)
        se = np.full(128, -1.0, np.float32)
        mask_x = toks_extra >= 0
        se[mask_x] = seg_global[toks_extra[mask_x]] - s_lo
        segm[:, 128] = se

        in_maps.append({
            "xT": xT,
            "wih_f": wih[0], "wih_b": wih[1],
            "whh_f": whh[0], "whh_b": whh[1],
            "b4_f": b4[0], "b4_b": b4[1], "i4": i4,
            "h0f": h0f.astype(BF16), "c0f": c0f.astype(BF16),
            "h0b": h0b.astype(BF16), "c0b": c0b.astype(BF16),
            "wom": wom, "uo": uo, "iota": iota,
            "identb": identb, "identf": identf,
            "seg": segm,
        })
    return in_maps, s_los


def kernel(**inputs):
    global LAST_RESULT
    from concourse.bass_utils import run_bass_kernel_spmd

    nc = _build()
    in_maps, s_los = _host_prep(inputs)
    res = run_bass_kernel_spmd(nc, in_maps, core_ids=list(range(NCORE)))
    LAST_RESULT = res

    G = np.zeros((S + SWIN, 257), np.float64)
    for c in range(NCORE):
        ctx = np.asarray(res.results[c]["ctx"], np.float32)
        G[s_los[c]:s_los[c] + SWIN] += ctx
    G = G[:S]
    z = G[:, 256]
    ctx = G[:, :256] / np.where(z == 0, 1.0, z)[:, None]
    w_tag = np.asarray(inputs["w_tag"], np.float32)
    b_tag = np.asarray(inputs["b_tag"], np.float32)
    out = ctx.astype(np.float32) @ w_tag.T + b_tag
    return out.astype(np.float32)

